# revision 22
# baseline (speedup 1.0000x reference)
"""Trainium2 Bass kernel for CrossModalAttention2D.

Math note: the attention has kv_len == 1 (text is mean-pooled to a single
token), so softmax over the key axis is identically 1.0 and the attention
output for every query position equals v[b].  The LayerNorm + Q projection
therefore do not affect the output at all; the module reduces exactly to

    t[b]   = mean_n text[b, n, :]                      # (C,)
    p[b]   = ((t Wv^T + bv) out_w^T + out_b) proj_w^T + proj_b
    out    = visual + gamma * p[b][None, :, None, None]

which is what this kernel computes.  Sharding: data-parallel over B — core c
handles batch c.

The kernel is pure HBM-bandwidth-bound: per core 8 MB visual in + 8 MB out
(fp16; host-side dtype cast halves the fp32 traffic) + 3 MB fp8 chain
weights = 19 MB at the ~358 GB/s per-core HBM limit.  Design notes, all
HW-measured:
  - per-DMA-queue delivery is ~1/3 of fabric rate and schedule granularity
    beats per-DMA efficiency: visual moves in 1 MB [128, HW] chunks;
  - chain weight halves (k-chunks 0-3 / 4-7) ride at the HEAD of both
    HWDGE queues while SWDGE (Q7 boots ~9 us late) carries pure visual;
  - the chain runs fp8 DoubleRow matmuls (2 k-tiles per pass; the
    stationary pair-dim must be stride 16 -> padded activation layout);
  - adds all on DVE; stores HWDGE-only in add-completion order, final
    chunk's store split across both queues to drain the tail at full rate.
Measured end-to-end rel err vs the fp32 reference: ~5e-3 (gate: 2e-2).
"""

import os
import sys

sys.path.insert(0, "/opt/trn_rl_repo")

import numpy as np

import concourse.bass as bass
import concourse.mybir as mybir
from concourse.tile import TileContext
from concourse.bass_utils import run_bass_kernel_spmd

B, C, H, W, NH, NT = 8, 1024, 64, 64, 16, 8
HW = H * W
P = 128
NCH = C // P  # 8 channel chunks
F32 = mybir.dt.float32
F16 = mybir.dt.float16
BF16 = mybir.dt.bfloat16
F8 = mybir.dt.float8e4
WDT = F8
PADW = 16     # DoubleRow stationary pair stride (ISA: step%16==0)
HALF = C // 2
HC = NCH * C // 2


def _split_waits(nc):
    """walrus in this env accepts at most ONE sync-wait per instruction.
    Hoist extra waits onto NoOps inserted just before, on the same engine
    (per-engine program order makes this semantically identical)."""
    for fn in nc.m.functions:
        for blk in fn.blocks:
            rebuilt = []
            changed = False
            for inst in blk.instructions:
                si = inst.sync_info
                if si is not None and si.on_wait is not None and len(si.on_wait) > 1:
                    waits = list(si.on_wait)
                    for i, w in enumerate(waits[:-1]):
                        rebuilt.append(
                            mybir.InstNoOp(
                                name=f"{inst.name}-ws{i}",
                                engine=inst.engine,
                                sync_info=mybir.SyncInfo(on_wait=[w], on_update=[]),
                                bass_nofuse=True,
                            )
                        )
                    si.on_wait = [waits[-1]]
                    changed = True
                rebuilt.append(inst)
            if changed:
                blk.instructions = rebuilt


def _build_nc():
    nc = bass.Bass(trn_type="TRN2")

    vis = nc.dram_tensor("vis", [C, HW], F16, kind="ExternalInput")
    wv_sb = nc.dram_tensor("wv_sb", [P, NCH * C], WDT, kind="ExternalInput")
    wo_sb = nc.dram_tensor("wo_sb", [P, NCH * C], WDT, kind="ExternalInput")
    wp_sb = nc.dram_tensor("wp_sb", [P, NCH * C], WDT, kind="ExternalInput")
    colblob = nc.dram_tensor("colblob", [P, NCH * NT + NCH + 1], F32,
                             kind="ExternalInput")
    rowblob = nc.dram_tensor("rowblob", [1, 2 * C], F32, kind="ExternalInput")
    out = nc.dram_tensor("out", [C, HW], F16, kind="ExternalOutput")

    with TileContext(nc) as tc:
        with (
            tc.tile_pool(name="cols", bufs=1) as cols,
            tc.tile_pool(name="wpool", bufs=6) as wpool,
            tc.tile_pool(name="psum", bufs=NCH, space="PSUM") as psum,
            tc.tile_pool(name="vispool", bufs=NCH) as vispool,
        ):
            identb = cols.tile([1, 1], BF16, tag="identb")
            identf = cols.tile([1, 1], F32, tag="identf")
            nc.vector.memset(identb, 1.0)
            nc.vector.memset(identf, 1.0)

            CBW = NCH * NT + NCH + 1
            cb = cols.tile([P, CBW], F32, tag="cb")
            nc.scalar.dma_start(out=cb, in_=colblob[:, :])
            rb = cols.tile([1, 2 * C], F32, tag="rb")
            nc.scalar.dma_start(out=rb, in_=rowblob[:, :])
            tt = cb[:, : NCH * NT].rearrange("p (k n) -> p k n", k=NCH)
            bpc = cb[:, NCH * NT : NCH * NT + NCH]
            gc = cb[:, NCH * NT + NCH : CBW]
            bvr = rb[:, :C]
            bor = rb[:, C:]

            # ---- weight halves lead BOTH HWDGE queues.  Per-queue delivery
            # is only ~1/3 of fabric rate once all queues compete, so a
            # weight buried behind visual arrives 30us+ late (measured: an
            # 8us full-fabric stall while every store waited on gp).  Half
            # tiles on two queues put all 3 matrices on-chip by ~16us while
            # SWDGE (booting ~9us) starts the visual flood — fabric stays
            # saturated throughout. ----
            wts = {}
            for name in ("v", "u", "g"):
                wts[name] = [
                    wpool.tile([P, HC], WDT, tag=f"wt{name}{h}", bufs=1,
                               name=f"wt{name}{h}")
                    for h in range(2)
                ]
            vts = {m: vispool.tile([P, HW], F16, tag="vt", name=f"vt{m}")
                   for m in range(NCH)}

            def vload(eng, m):
                eng.dma_start(out=vts[m], in_=vis[m * P : (m + 1) * P, :])

            for m in (4, 5, 6, 7):
                vload(nc.gpsimd, m)
            # interleave one visual load between successive weight halves:
            # weights still land by ~27us (chain needs wp only at ~26), and
            # HWDGE visual bytes start flowing ~7us earlier
            nc.sync.dma_start(out=wts["v"][0], in_=wv_sb[:, :HC])
            nc.scalar.dma_start(out=wts["v"][1], in_=wv_sb[:, HC:])
            vload(nc.sync, 0)
            vload(nc.scalar, 1)
            nc.sync.dma_start(out=wts["u"][0], in_=wo_sb[:, :HC])
            nc.scalar.dma_start(out=wts["u"][1], in_=wo_sb[:, HC:])
            vload(nc.sync, 2)
            vload(nc.scalar, 3)
            nc.sync.dma_start(out=wts["g"][0], in_=wp_sb[:, :HC])
            nc.scalar.dma_start(out=wts["g"][1], in_=wp_sb[:, HC:])

            # ---- t = mean(text), written into the padded stride-16 column
            # layout DoubleRow's LDWEIGHTS requires ----
            tsum = cols.tile([P, NCH], F32, tag="tsum")
            nc.vector.reduce_sum(tsum, tt, axis=mybir.AxisListType.X)
            nc.vector.tensor_scalar_mul(tsum, tsum, 1.0 / NT)
            tb = cols.tile([P, PADW * NCH], WDT, tag="tb")
            nc.vector.tensor_copy(
                tb.rearrange("p (k s) -> p k s", s=PADW)[:, :, 0], tsum)

            # ---- 3-layer chain: fp8 DoubleRow row-major matmuls (2 k-tiles
            # per pass), row result transposed back to columns via K=1 PE
            # transposes ----
            def layer(in_tile, whalves, bias_row, name):
                last = name == "g"
                psr = [psum.tile([1, HALF], F32, tag="ps", name=f"psr_{name}{h}")
                       for h in range(2)]
                wks = [wh.rearrange("p (k c) -> p k c", k=NCH // 2)
                       for wh in whalves]
                for kp in range(NCH // 2):
                    lhsT = in_tile[:, 2 * PADW * kp : 2 * PADW * (kp + 1)]\
                        .rearrange("p (two s) -> p two s", two=2)[:, :, 0]
                    wk = wks[kp // 2]
                    kl = 2 * (kp % 2)
                    for h in range(2):
                        nc.tensor.matmul(
                            psr[h], lhsT,
                            wk[:, kl : kl + 2, h * HALF : (h + 1) * HALF],
                            start=(kp == 0),
                            stop=(kp == NCH // 2 - 1),
                            perf_mode=mybir.MatmulPerfMode.DoubleRow,
                        )
                row = cols.tile([1, C], F32 if last else BF16, tag=f"row{name}")
                for h in range(2):
                    sl = slice(h * HALF, (h + 1) * HALF)
                    if last:
                        nc.vector.tensor_copy(row[:, sl], psr[h])
                    else:
                        nc.vector.tensor_tensor(
                            out=row[:, sl], in0=psr[h], in1=bias_row[:, sl],
                            op=mybir.AluOpType.add,
                        )
                out_tile = cols.tile([P, NCH if last else PADW * NCH],
                                     F32 if last else WDT, tag=f"oc{name}")
                for mo in range(NCH):
                    pc = psum.tile([P, 1], F32 if last else BF16, tag="ps",
                                   name=f"psT_{name}{mo}")
                    nc.tensor.transpose(
                        pc, row[:, mo * P : (mo + 1) * P],
                        identf if last else identb,
                    )
                    if last:
                        # gp = (p + bias) * gamma
                        nc.vector.tensor_scalar(
                            out_tile[:, mo : mo + 1], pc, bpc[:, mo : mo + 1],
                            gc[:, 0:1],
                            op0=mybir.AluOpType.add, op1=mybir.AluOpType.mult,
                        )
                    else:
                        nc.vector.tensor_copy(
                            out_tile[:, PADW * mo : PADW * mo + 1], pc)
                return out_tile

            vtile = layer(tb, wts["v"], bvr, "v")
            utile = layer(vtile, wts["u"], bor, "u")
            gp = layer(utile, wts["g"], None, "g")

            # ---- adds all on DVE (fp16 packed mode, ~1.2us per tile vs
            # 3.6us on ACT; and DVE pushes no DMA queue, so adds never delay
            # a store push).  Ordered by expected tile arrival. ----
            ADD_ORDER = [0, 1, 4, 2, 3, 5, 6, 7]
            for m in ADD_ORDER:
                nc.vector.tensor_scalar_add(vts[m], vts[m], gp[:, m : m + 1])

            # ---- stores: HWDGE only (fast push path), per-engine push order
            # matches add-completion order; the final chunk's store is split
            # across both queues so the tail drains at full fabric rate ----
            for m in (0, 2, 5, 7):
                if m == 7:
                    nc.sync.dma_start(out=out[m * P : (m + 1) * P, : HW // 2],
                                      in_=vts[m][:, : HW // 2])
                else:
                    nc.sync.dma_start(out=out[m * P : (m + 1) * P, :],
                                      in_=vts[m])
            for m in (1, 4, 3, 6, 7):
                if m == 7:
                    nc.scalar.dma_start(out=out[m * P : (m + 1) * P, HW // 2 :],
                                        in_=vts[m][:, HW // 2 :])
                else:
                    nc.scalar.dma_start(out=out[m * P : (m + 1) * P, :],
                                        in_=vts[m])

    _split_waits(nc)
    return nc


def _install_ntff_hook():
    """This container's `antenv` stub lacks axon_hooks; recreate the NTFF
    profiling hook via ctypes against the axon PJRT .so (same logic as
    trn_agent_boot.trn_boot)."""
    try:
        from antenv.axon_hooks import get_axon_ntff_profile_hook  # noqa: F401
        return
    except ImportError:
        pass
    import contextlib
    import ctypes
    import types

    so_path = "/opt/axon/libaxon_pjrt.so"
    if not os.path.exists(so_path):
        return
    lib = ctypes.CDLL(so_path)
    if not hasattr(lib, "axon_start_nrt_profile"):
        return
    lib.axon_start_nrt_profile.argtypes = [
        ctypes.POINTER(ctypes.c_int64), ctypes.c_size_t,
    ]
    lib.axon_start_nrt_profile.restype = ctypes.c_int64
    lib.axon_stop_nrt_profile.argtypes = [ctypes.c_char_p]
    lib.axon_stop_nrt_profile.restype = ctypes.c_int64

    @contextlib.contextmanager
    def _hook(output_dir, device_ids):
        import jax

        jax.devices()
        if device_ids:
            ids = (ctypes.c_int64 * len(device_ids))(*device_ids)
            rc = lib.axon_start_nrt_profile(ids, len(device_ids))
        else:
            rc = lib.axon_start_nrt_profile(None, 0)
        if rc != 0:
            raise RuntimeError(f"axon_start_nrt_profile rc={rc}")
        try:
            yield
        finally:
            n = lib.axon_stop_nrt_profile(str(output_dir).encode())
            print(f"ntff profile: {n} file(s) written to {output_dir}")

    import antenv

    mod = types.ModuleType("antenv.axon_hooks")
    mod.get_axon_ntff_profile_hook = lambda: _hook
    mod.set_axon_ntff_profile_hook = lambda h: None
    sys.modules["antenv.axon_hooks"] = mod
    antenv.axon_hooks = mod


_NC_CACHE = {}


def _get_nc():
    if "nc" not in _NC_CACHE:
        _NC_CACHE["nc"] = _build_nc()
    return _NC_CACHE["nc"]


def kernel(visual, text, in_proj_w, in_proj_b, out_w, out_b, ln_w, ln_b,
           proj_w, proj_b, gamma):
    visual = np.asarray(visual, dtype=np.float32)
    text = np.asarray(text, dtype=np.float32)
    in_proj_w = np.asarray(in_proj_w, dtype=np.float32)
    in_proj_b = np.asarray(in_proj_b, dtype=np.float32)

    # host-side input marshalling (layout/dtype only, no math)
    import ml_dtypes

    wdt = ml_dtypes.float8_e4m3fn

    def sb_layout(wT):
        # wT is [c, j]; SBUF layout row p = [wT[0*P+p, :], wT[1*P+p, :], ...]
        return np.ascontiguousarray(
            wT.reshape(NCH, P, C).transpose(1, 0, 2).reshape(P, NCH * C)
        ).astype(wdt)

    wv_sb = sb_layout(in_proj_w[2 * C : 3 * C].T)
    wo_sb = sb_layout(np.asarray(out_w, dtype=np.float32).T)
    wp_sb = sb_layout(np.asarray(proj_w, dtype=np.float32).T)
    rowblob = np.concatenate([
        in_proj_b[2 * C : 3 * C],
        np.asarray(out_b, dtype=np.float32),
    ]).reshape(1, 2 * C)
    bp_col = np.asarray(proj_b, dtype=np.float32).reshape(NCH, P).T  # (P, NCH)
    gamma_col = np.full((P, 1), np.asarray(gamma, dtype=np.float32).reshape(-1)[0],
                        dtype=np.float32)

    # fp16 visual: halves the dominant HBM traffic (dtype cast only)
    vis2d = np.ascontiguousarray(visual.reshape(B, C, HW).astype(np.float16))
    in_maps = []
    for c in range(B):
        ttc = text[c].T.reshape(NCH, P, NT).transpose(1, 0, 2).reshape(P, NCH * NT)
        colblob = np.ascontiguousarray(
            np.concatenate([ttc, bp_col, gamma_col], axis=1), dtype=np.float32)
        in_maps.append({
            "vis": vis2d[c],
            "wv_sb": wv_sb, "wo_sb": wo_sb, "wp_sb": wp_sb,
            "colblob": colblob, "rowblob": rowblob,
        })

    nc = _get_nc()
    trace = os.environ.get("BASS_KERNEL_TRACE", "") == "1"
    if trace:
        _install_ntff_hook()
    try:
        res = run_bass_kernel_spmd(nc, in_maps, core_ids=list(range(B)), trace=trace)
    except Exception:
        # transient NRT device errors have been observed once in ~15 runs;
        # one retry recovers
        res = run_bass_kernel_spmd(nc, in_maps, core_ids=list(range(B)), trace=trace)
    if trace:
        _NC_CACHE["last_results"] = res

    out = np.empty((B, C, HW), dtype=np.float32)
    for c in range(B):
        out[c] = res.results[c]["out"]
    return out.reshape(B, C, H, W)


# revision 23
# speedup vs baseline: 1.0053x; 1.0053x over previous
"""Trainium2 Bass kernel for CrossModalAttention2D.

Math note: the attention has kv_len == 1 (text is mean-pooled to a single
token), so softmax over the key axis is identically 1.0 and the attention
output for every query position equals v[b].  The LayerNorm + Q projection
therefore do not affect the output at all; the module reduces exactly to

    t[b]   = mean_n text[b, n, :]                      # (C,)
    p[b]   = ((t Wv^T + bv) out_w^T + out_b) proj_w^T + proj_b
    out    = visual + gamma * p[b][None, :, None, None]

which is what this kernel computes.  Sharding: data-parallel over B — core c
handles batch c.

The kernel is pure HBM-bandwidth-bound: per core 8 MB visual in + 8 MB out
(fp16; host-side dtype cast halves the fp32 traffic) + 3 MB fp8 chain
weights = 19 MB at the ~358 GB/s per-core HBM limit.  Design notes, all
HW-measured:
  - per-DMA-queue delivery is ~1/3 of fabric rate and schedule granularity
    beats per-DMA efficiency: visual moves in 1 MB [128, HW] chunks;
  - chain weight halves (k-chunks 0-3 / 4-7) ride at the HEAD of both
    HWDGE queues while SWDGE (Q7 boots ~9 us late) carries pure visual;
  - the chain runs fp8 DoubleRow matmuls (2 k-tiles per pass; the
    stationary pair-dim must be stride 16 -> padded activation layout);
  - adds all on DVE; stores HWDGE-only in add-completion order, final
    chunk's store split across both queues to drain the tail at full rate.
Measured end-to-end rel err vs the fp32 reference: ~5e-3 (gate: 2e-2).
"""

import os
import sys

sys.path.insert(0, "/opt/trn_rl_repo")

import numpy as np

import concourse.bass as bass
import concourse.mybir as mybir
from concourse.tile import TileContext
from concourse.bass_utils import run_bass_kernel_spmd

B, C, H, W, NH, NT = 8, 1024, 64, 64, 16, 8
HW = H * W
P = 128
NCH = C // P  # 8 channel chunks
F32 = mybir.dt.float32
F16 = mybir.dt.float16
BF16 = mybir.dt.bfloat16
F8 = mybir.dt.float8e4
WDT = F8
PADW = 16     # DoubleRow stationary pair stride (ISA: step%16==0)
HALF = C // 2
HC = NCH * C // 2


def _split_waits(nc):
    """walrus in this env accepts at most ONE sync-wait per instruction.
    Hoist extra waits onto NoOps inserted just before, on the same engine
    (per-engine program order makes this semantically identical)."""
    for fn in nc.m.functions:
        for blk in fn.blocks:
            rebuilt = []
            changed = False
            for inst in blk.instructions:
                si = inst.sync_info
                if si is not None and si.on_wait is not None and len(si.on_wait) > 1:
                    waits = list(si.on_wait)
                    for i, w in enumerate(waits[:-1]):
                        rebuilt.append(
                            mybir.InstNoOp(
                                name=f"{inst.name}-ws{i}",
                                engine=inst.engine,
                                sync_info=mybir.SyncInfo(on_wait=[w], on_update=[]),
                                bass_nofuse=True,
                            )
                        )
                    si.on_wait = [waits[-1]]
                    changed = True
                rebuilt.append(inst)
            if changed:
                blk.instructions = rebuilt


def _build_nc():
    nc = bass.Bass(trn_type="TRN2")

    vis = nc.dram_tensor("vis", [C, HW], F16, kind="ExternalInput")
    wv_sb = nc.dram_tensor("wv_sb", [P, NCH * C], WDT, kind="ExternalInput")
    wo_sb = nc.dram_tensor("wo_sb", [P, NCH * C], WDT, kind="ExternalInput")
    wp_sb = nc.dram_tensor("wp_sb", [P, NCH * C], WDT, kind="ExternalInput")
    colblob = nc.dram_tensor("colblob", [P, NCH * NT + NCH + 1], F32,
                             kind="ExternalInput")
    rowblob = nc.dram_tensor("rowblob", [1, 2 * C], F32, kind="ExternalInput")
    out = nc.dram_tensor("out", [C, HW], F16, kind="ExternalOutput")

    with TileContext(nc) as tc:
        with (
            tc.tile_pool(name="cols", bufs=1) as cols,
            tc.tile_pool(name="wpool", bufs=6) as wpool,
            tc.tile_pool(name="psum", bufs=NCH, space="PSUM") as psum,
            tc.tile_pool(name="vispool", bufs=NCH) as vispool,
        ):
            identb = cols.tile([1, 1], BF16, tag="identb")
            identf = cols.tile([1, 1], F32, tag="identf")
            nc.vector.memset(identb, 1.0)
            nc.vector.memset(identf, 1.0)

            CBW = NCH * NT + NCH + 1
            cb = cols.tile([P, CBW], F32, tag="cb")
            nc.scalar.dma_start(out=cb, in_=colblob[:, :])
            rb = cols.tile([1, 2 * C], F32, tag="rb")
            nc.scalar.dma_start(out=rb, in_=rowblob[:, :])
            tt = cb[:, : NCH * NT].rearrange("p (k n) -> p k n", k=NCH)
            bpc = cb[:, NCH * NT : NCH * NT + NCH]
            gc = cb[:, NCH * NT + NCH : CBW]
            bvr = rb[:, :C]
            bor = rb[:, C:]

            # ---- weight halves lead BOTH HWDGE queues.  Per-queue delivery
            # is only ~1/3 of fabric rate once all queues compete, so a
            # weight buried behind visual arrives 30us+ late (measured: an
            # 8us full-fabric stall while every store waited on gp).  Half
            # tiles on two queues put all 3 matrices on-chip by ~16us while
            # SWDGE (booting ~9us) starts the visual flood — fabric stays
            # saturated throughout. ----
            wts = {}
            for name in ("v", "u", "g"):
                wts[name] = [
                    wpool.tile([P, HC], WDT, tag=f"wt{name}{h}", bufs=1,
                               name=f"wt{name}{h}")
                    for h in range(2)
                ]
            for name, dram in (("v", wv_sb), ("u", wo_sb), ("g", wp_sb)):
                nc.sync.dma_start(out=wts[name][0], in_=dram[:, :HC])
                nc.scalar.dma_start(out=wts[name][1], in_=dram[:, HC:])

            vts = {m: vispool.tile([P, HW], F16, tag="vt", name=f"vt{m}")
                   for m in range(NCH)}

            def vload(eng, m):
                eng.dma_start(out=vts[m], in_=vis[m * P : (m + 1) * P, :])

            for m in (4, 5, 6, 7):
                vload(nc.gpsimd, m)
            vload(nc.sync, 0)
            vload(nc.sync, 2)
            vload(nc.scalar, 1)
            vload(nc.scalar, 3)

            # ---- t = mean(text), written into the padded stride-16 column
            # layout DoubleRow's LDWEIGHTS requires ----
            tsum = cols.tile([P, NCH], F32, tag="tsum")
            nc.vector.reduce_sum(tsum, tt, axis=mybir.AxisListType.X)
            nc.vector.tensor_scalar_mul(tsum, tsum, 1.0 / NT)
            tb = cols.tile([P, PADW * NCH], WDT, tag="tb")
            nc.vector.tensor_copy(
                tb.rearrange("p (k s) -> p k s", s=PADW)[:, :, 0], tsum)

            # ---- 3-layer chain: fp8 DoubleRow row-major matmuls (2 k-tiles
            # per pass), row result transposed back to columns via K=1 PE
            # transposes ----
            def layer(in_tile, whalves, bias_row, name):
                last = name == "g"
                psr = [psum.tile([1, HALF], F32, tag="ps", name=f"psr_{name}{h}")
                       for h in range(2)]
                wks = [wh.rearrange("p (k c) -> p k c", k=NCH // 2)
                       for wh in whalves]
                for kp in range(NCH // 2):
                    lhsT = in_tile[:, 2 * PADW * kp : 2 * PADW * (kp + 1)]\
                        .rearrange("p (two s) -> p two s", two=2)[:, :, 0]
                    wk = wks[kp // 2]
                    kl = 2 * (kp % 2)
                    for h in range(2):
                        nc.tensor.matmul(
                            psr[h], lhsT,
                            wk[:, kl : kl + 2, h * HALF : (h + 1) * HALF],
                            start=(kp == 0),
                            stop=(kp == NCH // 2 - 1),
                            perf_mode=mybir.MatmulPerfMode.DoubleRow,
                        )
                row = cols.tile([1, C], F32 if last else BF16, tag=f"row{name}")
                for h in range(2):
                    sl = slice(h * HALF, (h + 1) * HALF)
                    if last:
                        nc.vector.tensor_copy(row[:, sl], psr[h])
                    else:
                        nc.vector.tensor_tensor(
                            out=row[:, sl], in0=psr[h], in1=bias_row[:, sl],
                            op=mybir.AluOpType.add,
                        )
                out_tile = cols.tile([P, NCH if last else PADW * NCH],
                                     F32 if last else WDT, tag=f"oc{name}")
                for mo in range(NCH):
                    pc = psum.tile([P, 1], F32 if last else BF16, tag="ps",
                                   name=f"psT_{name}{mo}")
                    nc.tensor.transpose(
                        pc, row[:, mo * P : (mo + 1) * P],
                        identf if last else identb,
                    )
                    if last:
                        # gp = (p + bias) * gamma
                        nc.vector.tensor_scalar(
                            out_tile[:, mo : mo + 1], pc, bpc[:, mo : mo + 1],
                            gc[:, 0:1],
                            op0=mybir.AluOpType.add, op1=mybir.AluOpType.mult,
                        )
                    else:
                        nc.vector.tensor_copy(
                            out_tile[:, PADW * mo : PADW * mo + 1], pc)
                return out_tile

            vtile = layer(tb, wts["v"], bvr, "v")
            utile = layer(vtile, wts["u"], bor, "u")
            gp = layer(utile, wts["g"], None, "g")

            # ---- adds all on DVE (fp16 packed mode, ~1.2us per tile vs
            # 3.6us on ACT; and DVE pushes no DMA queue, so adds never delay
            # a store push).  Ordered by expected tile arrival. ----
            ADD_ORDER = [0, 1, 4, 2, 3, 5, 6, 7]
            for m in ADD_ORDER:
                nc.vector.tensor_scalar_add(vts[m], vts[m], gp[:, m : m + 1])

            # ---- stores: HWDGE only (fast push path), per-engine push order
            # matches add-completion order; the final chunk's store is split
            # across both queues so the tail drains at full fabric rate ----
            for m in (0, 2, 5, 7):
                if m == 7:
                    nc.sync.dma_start(out=out[m * P : (m + 1) * P, : HW // 2],
                                      in_=vts[m][:, : HW // 2])
                else:
                    nc.sync.dma_start(out=out[m * P : (m + 1) * P, :],
                                      in_=vts[m])
            for m in (1, 4, 3, 6, 7):
                if m == 7:
                    nc.scalar.dma_start(out=out[m * P : (m + 1) * P, HW // 2 :],
                                        in_=vts[m][:, HW // 2 :])
                else:
                    nc.scalar.dma_start(out=out[m * P : (m + 1) * P, :],
                                        in_=vts[m])

    _split_waits(nc)
    return nc


def _install_ntff_hook():
    """This container's `antenv` stub lacks axon_hooks; recreate the NTFF
    profiling hook via ctypes against the axon PJRT .so (same logic as
    trn_agent_boot.trn_boot)."""
    try:
        from antenv.axon_hooks import get_axon_ntff_profile_hook  # noqa: F401
        return
    except ImportError:
        pass
    import contextlib
    import ctypes
    import types

    so_path = "/opt/axon/libaxon_pjrt.so"
    if not os.path.exists(so_path):
        return
    lib = ctypes.CDLL(so_path)
    if not hasattr(lib, "axon_start_nrt_profile"):
        return
    lib.axon_start_nrt_profile.argtypes = [
        ctypes.POINTER(ctypes.c_int64), ctypes.c_size_t,
    ]
    lib.axon_start_nrt_profile.restype = ctypes.c_int64
    lib.axon_stop_nrt_profile.argtypes = [ctypes.c_char_p]
    lib.axon_stop_nrt_profile.restype = ctypes.c_int64

    @contextlib.contextmanager
    def _hook(output_dir, device_ids):
        import jax

        jax.devices()
        if device_ids:
            ids = (ctypes.c_int64 * len(device_ids))(*device_ids)
            rc = lib.axon_start_nrt_profile(ids, len(device_ids))
        else:
            rc = lib.axon_start_nrt_profile(None, 0)
        if rc != 0:
            raise RuntimeError(f"axon_start_nrt_profile rc={rc}")
        try:
            yield
        finally:
            n = lib.axon_stop_nrt_profile(str(output_dir).encode())
            print(f"ntff profile: {n} file(s) written to {output_dir}")

    import antenv

    mod = types.ModuleType("antenv.axon_hooks")
    mod.get_axon_ntff_profile_hook = lambda: _hook
    mod.set_axon_ntff_profile_hook = lambda h: None
    sys.modules["antenv.axon_hooks"] = mod
    antenv.axon_hooks = mod


_NC_CACHE = {}


def _get_nc():
    if "nc" not in _NC_CACHE:
        _NC_CACHE["nc"] = _build_nc()
    return _NC_CACHE["nc"]


def kernel(visual, text, in_proj_w, in_proj_b, out_w, out_b, ln_w, ln_b,
           proj_w, proj_b, gamma):
    visual = np.asarray(visual, dtype=np.float32)
    text = np.asarray(text, dtype=np.float32)
    in_proj_w = np.asarray(in_proj_w, dtype=np.float32)
    in_proj_b = np.asarray(in_proj_b, dtype=np.float32)

    # host-side input marshalling (layout/dtype only, no math)
    import ml_dtypes

    wdt = ml_dtypes.float8_e4m3fn

    def sb_layout(wT):
        # wT is [c, j]; SBUF layout row p = [wT[0*P+p, :], wT[1*P+p, :], ...]
        return np.ascontiguousarray(
            wT.reshape(NCH, P, C).transpose(1, 0, 2).reshape(P, NCH * C)
        ).astype(wdt)

    wv_sb = sb_layout(in_proj_w[2 * C : 3 * C].T)
    wo_sb = sb_layout(np.asarray(out_w, dtype=np.float32).T)
    wp_sb = sb_layout(np.asarray(proj_w, dtype=np.float32).T)
    rowblob = np.concatenate([
        in_proj_b[2 * C : 3 * C],
        np.asarray(out_b, dtype=np.float32),
    ]).reshape(1, 2 * C)
    bp_col = np.asarray(proj_b, dtype=np.float32).reshape(NCH, P).T  # (P, NCH)
    gamma_col = np.full((P, 1), np.asarray(gamma, dtype=np.float32).reshape(-1)[0],
                        dtype=np.float32)

    # fp16 visual: halves the dominant HBM traffic (dtype cast only)
    vis2d = np.ascontiguousarray(visual.reshape(B, C, HW).astype(np.float16))
    in_maps = []
    for c in range(B):
        ttc = text[c].T.reshape(NCH, P, NT).transpose(1, 0, 2).reshape(P, NCH * NT)
        colblob = np.ascontiguousarray(
            np.concatenate([ttc, bp_col, gamma_col], axis=1), dtype=np.float32)
        in_maps.append({
            "vis": vis2d[c],
            "wv_sb": wv_sb, "wo_sb": wo_sb, "wp_sb": wp_sb,
            "colblob": colblob, "rowblob": rowblob,
        })

    nc = _get_nc()
    trace = os.environ.get("BASS_KERNEL_TRACE", "") == "1"
    if trace:
        _install_ntff_hook()
    try:
        res = run_bass_kernel_spmd(nc, in_maps, core_ids=list(range(B)), trace=trace)
    except Exception:
        # transient NRT device errors have been observed once in ~15 runs;
        # one retry recovers
        res = run_bass_kernel_spmd(nc, in_maps, core_ids=list(range(B)), trace=trace)
    if trace:
        _NC_CACHE["last_results"] = res

    out = np.empty((B, C, HW), dtype=np.float32)
    for c in range(B):
        out[c] = res.results[c]["out"]
    return out.reshape(B, C, H, W)
